# revision 20
# baseline (speedup 1.0000x reference)
"""Trainium2 Bass kernel for nn_BlockDecomposition (relational GNN message passing).

out[n] = sum_r sum_{e: type=r, tgt=n} w_e * (x[src_e] @ BD(blocks[r]))

Sharding: relation r -> core r (R == n_cores == 8). Each core:
  - gathers x[src] rows DIRECTLY from DRAM (no x@W pre-pass, no serialization)
  - edges packed into target windows of 128 relabeled targets; targets are
    degree-class sorted so window chunk counts match edge counts (~0.5% pad
    vs 50% for fixed chunks-per-window)
  - per chunk: one-hot(target-offset)*weight built on the Scalar engine
    (Square+Relu trick) keeping the DVE quiet -> no GpSimd SBUF-port
    contention with the Q7 descriptor generator (the critical path)
  - aggregation matmul in transposed orientation (aggT[64,128] PSUM), then
    the block-diagonal W applied per window on-chip: out_w = aggT^T @ W
Host: per-core target relabel inverse + sum of 8 per-relation partials.
"""
import numpy as np

N_NODES = 50000
N_PAD = 50048
D = 64
R = 8
P = 128
SPLIT = 32768            # source-rank split (int16 gather index limit)
HI_SZ = N_PAD - SPLIT    # 17280
BATCH_CH = 32            # chunks per dma_gather call (4096 indices)

_cache = {}


def _build_program(clo, chi):
    import concourse.bacc as bacc
    import concourse.bass as bass
    import concourse.tile as tile
    import concourse.mybir as mybir

    NW = len(clo)
    nch_lo = int(sum(clo))
    nch_hi = int(sum(chi))
    nch = nch_lo + nch_hi
    lo_start = np.concatenate([[0], np.cumsum(clo)]).astype(np.int64)
    hi_start = np.concatenate([[0], np.cumsum(chi)]).astype(np.int64)
    chunk_start = np.concatenate([[0], np.cumsum(np.asarray(clo) + np.asarray(chi))]).astype(np.int64)

    nc = bacc.Bacc("TRN2", target_bir_lowering=False, debug=False, num_devices=8,
                   num_swdge_queues=4)

    x_d = nc.dram_tensor("x", [N_PAD, D], mybir.dt.float32, kind="ExternalInput")
    il_d = nc.dram_tensor("il", [P, max(nch_lo, 1) * 8], mybir.dt.int16, kind="ExternalInput")
    ih_d = nc.dram_tensor("ih", [P, max(nch_hi, 1) * 8], mybir.dt.int16, kind="ExternalInput")
    wgt_d = nc.dram_tensor("wgt", [P, nch], mybir.dt.float32, kind="ExternalInput")
    tgt_d = nc.dram_tensor("tgt", [P, nch], mybir.dt.float32, kind="ExternalInput")
    nwgt_d = nc.dram_tensor("nwgt", [P, nch], mybir.dt.float32, kind="ExternalInput")
    ntgt_d = nc.dram_tensor("ntgt", [P, nch], mybir.dt.float32, kind="ExternalInput")
    iota_d = nc.dram_tensor("iota", [P, P], mybir.dt.float32, kind="ExternalInput")
    wbd_d = nc.dram_tensor("wbd", [D, D], mybir.dt.float32, kind="ExternalInput")
    out_d = nc.dram_tensor("out", [NW * P, D], mybir.dt.float32, kind="ExternalOutput")

    with tile.TileContext(nc) as tc:
        with (
            tc.tile_pool(name="consts", bufs=1) as consts,
            tc.tile_pool(name="edges", bufs=1) as edges,
            tc.tile_pool(name="msgs", bufs=5) as msgs_pool,
            tc.tile_pool(name="sq", bufs=4) as sq_pool,
            tc.tile_pool(name="oh", bufs=6) as oh_pool,
            tc.tile_pool(name="paggT", bufs=4, space="PSUM") as paggT,
            tc.tile_pool(name="pout", bufs=4, space="PSUM") as pout,
            tc.tile_pool(name="aggsb", bufs=4) as aggsb_pool,
            tc.tile_pool(name="evict", bufs=2) as evict_pool,
        ):
            il_t = edges.tile([P, max(nch_lo, 1) * 8], mybir.dt.int16, tag="il")
            ih_t = edges.tile([P, max(nch_hi, 1) * 8], mybir.dt.int16, tag="ih")
            wgt_t = edges.tile([P, nch], mybir.dt.float32, tag="wgt")
            tgt_t = edges.tile([P, nch], mybir.dt.float32, tag="tgt")
            nwgt_t = edges.tile([P, nch], mybir.dt.float32, tag="nwgt")
            ntgt_t = edges.tile([P, nch], mybir.dt.float32, tag="ntgt")
            # first gather's index slice loads first
            ncol = max(nch_lo, 1) * 8
            c0 = min(BATCH_CH * 8, ncol)
            nc.sync.dma_start(il_t[:, :c0], il_d[:, :c0])
            iota_f = consts.tile([P, P], mybir.dt.float32, tag="iota")
            nc.sync.dma_start(iota_f[:], iota_d[:])
            wbd_t = consts.tile([D, D], mybir.dt.float32, tag="wbd")
            nc.sync.dma_start(wbd_t[:], wbd_d[:])
            if ncol > c0:
                nc.sync.dma_start(il_t[:, c0:], il_d[:, c0:])
            nc.sync.dma_start(ih_t[:], ih_d[:])
            nc.sync.dma_start(wgt_t[:], wgt_d[:])
            nc.sync.dma_start(tgt_t[:], tgt_d[:])
            nc.sync.dma_start(nwgt_t[:], nwgt_d[:])
            nc.sync.dma_start(ntgt_t[:], ntgt_d[:])

            x_lo = x_d[0:SPLIT, :]
            x_hi = x_d[SPLIT:N_PAD, :]

            qrr = [0]

            def emit_gather(b, nch_s, idx_tile, src_ap, tag):
                ch = min(BATCH_CH, nch_s - b * BATCH_CH)
                ni = ch * P
                mt = msgs_pool.tile([P, BATCH_CH * D], mybir.dt.float32, tag=tag)
                nc.gpsimd.dma_gather(
                    out_ap=mt[:, :ch * D].rearrange("p (c e) -> p c e", e=D),
                    in_ap=src_ap,
                    idxs_ap=idx_tile[:, b * BATCH_CH * 8:b * BATCH_CH * 8 + ch * 8],
                    num_idxs=ni, num_idxs_reg=ni, elem_size=D,
                    single_packet=False, queue_num=qrr[0] % 4)
                qrr[0] += 1
                return mt

            # emit gathers in window-consumption order
            nb_lo = (nch_lo + BATCH_CH - 1) // BATCH_CH
            nb_hi = (nch_hi + BATCH_CH - 1) // BATCH_CH if nch_hi else 0
            ev = []
            for b in range(nb_lo):
                w = int(np.searchsorted(lo_start, b * BATCH_CH, side="right")) - 1
                ev.append((w, 0, b))
            for b in range(nb_hi):
                w = int(np.searchsorted(hi_start, b * BATCH_CH, side="right")) - 1
                ev.append((w, 1, b))
            ev.sort()
            lo_tiles, hi_tiles = {}, {}
            for _, s, b in ev:
                if s == 0:
                    lo_tiles[b] = emit_gather(b, nch_lo, il_t, x_lo, "mlo")
                else:
                    hi_tiles[b] = emit_gather(b, nch_hi, ih_t, x_hi, "mhi")

            out_bl = out_d[:].rearrange("(p w) e -> p (w e)", p=P)
            stg_box = [None]

            def flush(w, aggT_sb):
                ops = pout.tile([P, D], mybir.dt.float32, space="PSUM", tag="outp")
                nc.tensor.matmul(
                    out=ops[:], lhsT=aggT_sb[:], rhs=wbd_t[:],
                    start=True, stop=True)
                si = w % 8
                if si == 0:
                    stg_box[0] = evict_pool.tile(
                        [P, 8 * D], mybir.dt.float32, tag="ostg", name="ostg")
                stg = stg_box[0]
                nc.vector.tensor_copy(stg[:, si * D:(si + 1) * D], ops[:])
                if si == 7 or w == NW - 1:
                    w0 = w - si
                    nc.sync.dma_start(
                        out_bl[:, w0 * D:(w + 1) * D], stg[:, :(si + 1) * D])

            pend = None
            for w in range(NW):
                cpw = int(clo[w] + chi[w])
                ps = paggT.tile([D, P], mybir.dt.float32, space="PSUM", tag="aggT")
                for k in range(cpw):
                    j = int(chunk_start[w]) + k
                    if k < clo[w]:
                        js = int(lo_start[w]) + k
                        mt = lo_tiles[js // BATCH_CH]
                    else:
                        js = int(hi_start[w]) + (k - int(clo[w]))
                        mt = hi_tiles[js // BATCH_CH]
                    jl = js % BATCH_CH
                    oh = oh_pool.tile([P, P], mybir.dt.float32, tag="oh")
                    if j % 2 == 0:
                        sq = sq_pool.tile([P, P], mybir.dt.float32, tag="sq")
                        nc.scalar.activation(
                            out=sq[:], in_=iota_f[:],
                            func=mybir.ActivationFunctionType.Square,
                            bias=ntgt_t[:, j:j + 1], scale=1.0)
                        nc.scalar.activation(
                            out=oh[:], in_=sq[:],
                            func=mybir.ActivationFunctionType.Relu,
                            bias=wgt_t[:, j:j + 1], scale=nwgt_t[:, j:j + 1])
                    else:
                        nc.vector.tensor_scalar(
                            out=oh[:], in0=iota_f[:],
                            scalar1=tgt_t[:, j:j + 1], scalar2=wgt_t[:, j:j + 1],
                            op0=mybir.AluOpType.is_equal, op1=mybir.AluOpType.mult)
                    nc.tensor.matmul(
                        out=ps[:], lhsT=mt[:, jl * D:(jl + 1) * D], rhs=oh[:],
                        start=(k == 0), stop=(k == cpw - 1))
                aggT_sb = aggsb_pool.tile([D, P], mybir.dt.float32, tag="aggsb")
                nc.scalar.copy(aggT_sb[:], ps[:])
                if pend is not None:
                    flush(*pend)
                pend = (w, aggT_sb)
            if pend is not None:
                flush(*pend)

    nc.compile()
    return nc


def _pack_core(src, tgt, wgt):
    """Per-core packing. Returns (rank, worder, clo, chi, edge slot data)."""
    sdeg = np.bincount(src, minlength=N_PAD)
    order_s = np.argsort(-sdeg, kind="stable")
    rank = np.empty(N_PAD, np.int64)
    rank[order_s] = np.arange(N_PAD)
    src_rank = rank[src]
    is_hi = src_rank >= SPLIT

    deg_t = np.bincount(tgt, minlength=N_PAD)
    hi_t = np.bincount(tgt[is_hi], minlength=N_PAD)
    nz = np.where(deg_t > 0)[0]
    # degree-class sort (deg desc), hi-rich targets first within class
    key = deg_t[nz] * 100000 + hi_t[nz]
    worder = nz[np.argsort(-key, kind="stable")]
    NW = (len(worder) + P - 1) // P
    clo = np.zeros(NW, np.int64)
    chi = np.zeros(NW, np.int64)
    wo_pad = np.full(NW * P, -1, np.int64)
    wo_pad[:len(worder)] = worder
    wt = wo_pad.reshape(NW, P)
    for w in range(NW):
        m = wt[w] >= 0
        tw = wt[w][m]
        lo_cnt = int(deg_t[tw].sum() - hi_t[tw].sum())
        hi_cnt = int(hi_t[tw].sum())
        clo[w] = -(-lo_cnt // P) if lo_cnt else 0
        chi[w] = -(-hi_cnt // P) if hi_cnt else 0
    return rank, wo_pad, clo, chi, src_rank, is_hi, deg_t


def _streams_core(tgt, wgt, src_rank, is_hi, wo_pad, NW, clo, chi,
                  lo_start, hi_start, chunk_start, nch_lo, nch_hi, nch):
    """Build il/ih/wgt/nwgt/ntgt arrays for one core against the global profile."""
    tpos = np.full(N_PAD, -1, np.int64)
    val = wo_pad >= 0
    tpos[wo_pad[val]] = np.where(val)[0]
    ew = tpos[tgt] // P              # window of each edge
    eo = (tpos[tgt] % P).astype(np.float32)

    il = np.zeros(max(nch_lo, 1) * P, np.int16)
    ih = np.zeros(max(nch_hi, 1) * P, np.int16)
    wgt_arr = np.zeros((P, nch), np.float32)
    ntgt_arr = np.zeros((P, nch), np.float32)

    order = np.lexsort((is_hi, ew))
    ew_s = ew[order]
    eo_s = eo[order]
    hi_s = is_hi[order]
    rank_s = src_rank[order]
    wgt_s = wgt[order]
    starts = np.searchsorted(ew_s, np.arange(NW + 1))
    for w in range(NW):
        s0, s1 = int(starts[w]), int(starts[w + 1])
        hi_m = hi_s[s0:s1]
        for is_h, stream, st0, cap in (
            (False, il, int(lo_start[w]), int(clo[w]) * P),
            (True, ih, int(hi_start[w]), int(chi[w]) * P),
        ):
            sel = hi_m if is_h else ~hi_m
            n = int(sel.sum())
            assert n <= cap, (w, n, cap)
            if n == 0:
                continue
            rk = rank_s[s0:s1][sel] - (SPLIT if is_h else 0)
            stream[st0 * P:st0 * P + n] = rk.astype(np.int16)
            slots = np.arange(n)
            cw = slots // P
            lane = slots % P
            jg = int(chunk_start[w]) + (0 if not is_h else int(clo[w])) + cw
            wgt_arr[lane, jg] = wgt_s[s0:s1][sel]
            ntgt_arr[lane, jg] = eo_s[s0:s1][sel]

    def wrap(stream, nch_s):
        out = np.zeros((P, max(nch_s, 1) * 8), np.int16)
        nb = (nch_s + BATCH_CH - 1) // BATCH_CH
        for b in range(nb):
            ch = min(BATCH_CH, nch_s - b * BATCH_CH)
            seg = stream[b * BATCH_CH * P: b * BATCH_CH * P + ch * P]
            w16 = seg.reshape(ch * 8, 16).T
            out[:, b * BATCH_CH * 8: b * BATCH_CH * 8 + ch * 8] = np.tile(w16, (8, 1))
        return out

    return (wrap(il, nch_lo), wrap(ih, nch_hi), wgt_arr, ntgt_arr)


def kernel(x, blocks, edge_weights, source, target, edge_type):
    from concourse.bass_utils import run_bass_kernel_spmd

    x = np.asarray(x, np.float32)
    blocks = np.asarray(blocks, np.float32)
    edge_weights = np.asarray(edge_weights, np.float32)
    source = np.asarray(source, np.int64)
    target = np.asarray(target, np.int64)
    edge_type = np.asarray(edge_type, np.int64)

    n, d = x.shape
    assert n == N_NODES and d == D

    xp = np.zeros((N_PAD, D), np.float32)
    xp[:n] = x
    iota = np.broadcast_to(np.arange(P, dtype=np.float32), (P, P)).copy()

    # per-core packing
    packs = []
    for r in range(R):
        m = edge_type == r
        packs.append((_pack_core(source[m], target[m], edge_weights[m]),
                      source[m], target[m], edge_weights[m]))

    NW = max(len(p[0][2]) for p in packs)
    clo = np.zeros(NW, np.int64)
    chi = np.zeros(NW, np.int64)
    for (rank, wo, c1, c2, sr, ih_m, dt_), s_, t_, w_ in packs:
        clo[:len(c1)] = np.maximum(clo[:len(c1)], c1)
        chi[:len(c2)] = np.maximum(chi[:len(c2)], c2)

    nch_lo = int(clo.sum())
    nch_hi = int(chi.sum())
    nch = nch_lo + nch_hi
    lo_start = np.concatenate([[0], np.cumsum(clo)])
    hi_start = np.concatenate([[0], np.cumsum(chi)])
    chunk_start = np.concatenate([[0], np.cumsum(clo + chi)])

    key = (NW, tuple(clo.tolist()), tuple(chi.tolist()))
    if key not in _cache:
        _cache.clear()
        _cache[key] = _build_program(tuple(clo.tolist()), tuple(chi.tolist()))
    nc = _cache[key]

    in_maps = []
    for (rank, wo_pad, c1, c2, src_rank, is_hi, deg_t), s_, t_, w_ in packs:
        wopad_full = np.full(NW * P, -1, np.int64)
        wopad_full[:len(wo_pad)] = wo_pad
        il, ih, wgt_arr, tgt_arr = _streams_core(
            t_, w_, src_rank, is_hi, wopad_full, NW, clo, chi,
            lo_start, hi_start, chunk_start, nch_lo, nch_hi, nch)
        # per-core x permuted by source rank
        xr = np.zeros((N_PAD, D), np.float32)
        inv = np.empty(N_PAD, np.int64)
        inv[rank] = np.arange(N_PAD)
        xr = xp[inv]

        r_idx = len(in_maps)
        wbd = np.zeros((D, D), np.float32)
        bs = D // blocks.shape[1]
        for b in range(blocks.shape[1]):
            wbd[b * bs:(b + 1) * bs, b * bs:(b + 1) * bs] = blocks[r_idx, b]
        in_maps.append({
            "x": xr, "il": il, "ih": ih, "wgt": wgt_arr, "tgt": tgt_arr,
            "nwgt": -wgt_arr, "ntgt": -tgt_arr, "iota": iota, "wbd": wbd,
        })

    res = run_bass_kernel_spmd(nc, in_maps, core_ids=list(range(R)))

    out = np.zeros((N_PAD, D), np.float32)
    for r in range(R):
        bl = res.results[r]["out"].reshape(P, NW, D).transpose(1, 0, 2).reshape(NW * P, D)
        wo = packs[r][0][1]
        wopad_full = np.full(NW * P, -1, np.int64)
        wopad_full[:len(wo)] = wo
        m = wopad_full >= 0
        out[wopad_full[m]] += bl[m]
    return out[:N_NODES]
